# revision 8
# baseline (speedup 1.0000x reference)
"""Trainium2 Bass kernel for nn_CausalAttention_41961830482398.

Computes, for H,T [8192,512] and dim-512 linear layers Wq/Wk/Wv/Wo:
    dist  = pairwise_distances(T)                 # [N,N]
    scale = 1/(1 + mean(dist, axis=1))            # [N,1]
    Q,K,V = H@W{q,k,v}.T + b{q,k,v}
    attn  = softmax(Q@K.T / sqrt(512))
    out   = ((attn*scale) @ V) @ Wo.T + bo

Sharding: sequence-parallel over the row dim N across 8 cores (1024 rows
per core).  K, V and the projection weights are replicated (each core
computes full K,V from full H).  Everything is computed in a transposed
("S^T") layout so that no on-device transposes are needed:

  phase A: Kt = Wk@H^T + bk  [512,8192] (bounced via DRAM),
           V = H@Wv^T + bv   [8192,512] (SBUF-resident),
           Qt = Wq@Hs^T + bq [512,1024]
  phase B: G = Ts@T^T, dist = sqrt(max(tts+ttn-2G,0)+1e-8),
           row-mean via ACT accum -> scale_s  [1024]
  phase C: S^T tile = K@Qs^T  [n=128, m=512]; Pt = exp(S^T/sqrt(d));
           O^T += V^T@P^T via lhsT=V tiles; denom via ones-matmul;
           O^T *= scale_s/denom;  Out^T = Wo@O^T + bo -> DRAM.

Host passes pre-transposed/bf16-cast inputs and per-core shard slices;
the kernel returns Out^T per core which the host gathers + transposes.
"""

import numpy as np
import ml_dtypes

import concourse.bass as bass
import concourse.mybir as mybir
import concourse.tile as tile
from concourse import bacc
from concourse import bass_utils

N, DIM = 8192, 512
NCORES = 8
SHARD = N // NCORES          # 1024 rows per core
DC = DIM // 128              # 4 contraction chunks
NT = N // 128                # 64 key tiles
MBS = 512                    # m free-dim block
MBN = SHARD // MBS           # 2 m-blocks
NCH = N // 512               # 16 n chunks of 512
INV_SQRT_D = 1.0 / np.sqrt(np.float32(DIM))

BF16 = mybir.dt.bfloat16
F32 = mybir.dt.float32
AF = mybir.ActivationFunctionType
ALU = mybir.AluOpType
AX = mybir.AxisListType

bf16np = ml_dtypes.bfloat16


def build_kernel():
    nc = bacc.Bacc("TRN2", target_bir_lowering=False, debug=False)

    # ---- DRAM I/O ------------------------------------------------------
    Ht = nc.dram_tensor("Ht", [DIM, N], BF16, kind="ExternalInput")
    Hts = nc.dram_tensor("Hts", [DIM, SHARD], BF16, kind="ExternalInput")
    Tt = nc.dram_tensor("Tt", [DIM, N], BF16, kind="ExternalInput")
    Tts = nc.dram_tensor("Tts", [DIM, SHARD], BF16, kind="ExternalInput")
    ttn = nc.dram_tensor("ttn", [1, N], F32, kind="ExternalInput")
    tts = nc.dram_tensor("tts", [128, SHARD // 128], F32, kind="ExternalInput")
    WqT = nc.dram_tensor("WqT", [DIM, DIM], BF16, kind="ExternalInput")
    WkT = nc.dram_tensor("WkT", [DIM, DIM], BF16, kind="ExternalInput")
    WvT = nc.dram_tensor("WvT", [DIM, DIM], BF16, kind="ExternalInput")
    WoT = nc.dram_tensor("WoT", [DIM, DIM], BF16, kind="ExternalInput")
    bq_c = nc.dram_tensor("bq_c", [128, DC], F32, kind="ExternalInput")
    bk_c = nc.dram_tensor("bk_c", [128, DC], F32, kind="ExternalInput")
    bo_c = nc.dram_tensor("bo_c", [128, DC], F32, kind="ExternalInput")
    bv_r = nc.dram_tensor("bv_r", [1, DIM], F32, kind="ExternalInput")
    OutT = nc.dram_tensor("OutT", [DIM, SHARD], F32, kind="ExternalOutput")

    with tile.TileContext(nc) as tc:
        with (
            tc.tile_pool(name="dram", bufs=1, space="DRAM") as dpool,
            tc.tile_pool(name="singles", bufs=1) as sg,
        ):
            KtD = dpool.tile([DIM, N], BF16, tag="ktd")
            sscr = dpool.tile([MBN * 4, 128], F32, tag="sscr")
            oscr = dpool.tile([1, MBS], F32, tag="oscr")

            # long-lived SBUF tensors
            V_all = sg.tile([128, NT, DIM], BF16, tag="v")       # 64KB/part
            Qt_all = sg.tile([128, DC, SHARD], BF16, tag="qt")   # 8KB
            WoT_all = sg.tile([128, DC, DIM], BF16, tag="wot")   # 4KB
            bo_sb = sg.tile([128, DC], F32, tag="bo")
            ones_sb = sg.tile([128, 1], BF16, tag="ones")
            dist_acc = sg.tile([128, MBN * 4 * NCH], F32, tag="dacc")
            scale_col = sg.tile([128, MBN * 4], F32, tag="scol")
            scale_row = sg.tile([1, SHARD], F32, tag="srow")
            os_bc = sg.tile([128, MBS], F32, tag="osbc")

            nc.vector.memset(ones_sb, 1.0)
            nc.sync.dma_start(out=WoT_all, in_=WoT[:].rearrange("(c p) d -> p c d", p=128))
            nc.sync.dma_start(out=bo_sb, in_=bo_c[:])

            # ================= phase A: projections =====================
            with (
                tc.tile_pool(name="pa", bufs=1) as pa,
                tc.tile_pool(name="pa_st", bufs=4) as pa_st,
                tc.tile_pool(name="psum_a", bufs=3, space="PSUM") as ps_a,
            ):
                Ht_all = pa.tile([128, DC, N], BF16, tag="ht")
                Hts_all = pa.tile([128, DC, SHARD], BF16, tag="hts")
                Wq_sb = pa.tile([128, DC, DIM], BF16, tag="wq")
                Wk_sb = pa.tile([128, DC, DIM], BF16, tag="wk")
                Wv_sb = pa.tile([128, DC, DIM], BF16, tag="wv")
                bq_sb = pa.tile([128, DC], F32, tag="bq")
                bk_sb = pa.tile([128, DC], F32, tag="bk")
                bv_bc = pa.tile([128, DIM], F32, tag="bvbc")

                nc.sync.dma_start(out=Ht_all, in_=Ht[:].rearrange("(c p) n -> p c n", p=128))
                nc.sync.dma_start(out=Hts_all, in_=Hts[:].rearrange("(c p) n -> p c n", p=128))
                nc.sync.dma_start(out=Wq_sb, in_=WqT[:].rearrange("(c p) d -> p c d", p=128))
                nc.sync.dma_start(out=Wk_sb, in_=WkT[:].rearrange("(c p) d -> p c d", p=128))
                nc.sync.dma_start(out=Wv_sb, in_=WvT[:].rearrange("(c p) d -> p c d", p=128))
                nc.sync.dma_start(out=bq_sb, in_=bq_c[:])
                nc.sync.dma_start(out=bk_sb, in_=bk_c[:])
                bv_ap = bv_r[:]
                nc.sync.dma_start(
                    out=bv_bc,
                    in_=bass.AP(tensor=bv_ap.tensor, offset=bv_ap.offset,
                                ap=[[0, 128]] + bv_ap.ap[1:]),
                )

                # Qt = Wq @ Hs^T + bq   -> Qt_all [d-chunk, m]
                for d in range(DC):
                    for mb in range(MBN):
                        q_ps = ps_a.tile([128, MBS], F32, tag="a")
                        for e in range(DC):
                            nc.tensor.matmul(
                                q_ps,
                                Wq_sb[:, e, d * 128:(d + 1) * 128],
                                Hts_all[:, e, mb * MBS:(mb + 1) * MBS],
                                start=(e == 0), stop=(e == DC - 1),
                            )
                        nc.scalar.activation(
                            Qt_all[:, d, mb * MBS:(mb + 1) * MBS], q_ps,
                            AF.Identity, bias=bq_sb[:, d:d + 1], scale=1.0,
                        )

                # Kt = Wk @ H^T + bk -> DRAM bounce
                for d in range(DC):
                    for nch in range(NCH):
                        k_ps = ps_a.tile([128, 512], F32, tag="a")
                        for e in range(DC):
                            nc.tensor.matmul(
                                k_ps,
                                Wk_sb[:, e, d * 128:(d + 1) * 128],
                                Ht_all[:, e, nch * 512:(nch + 1) * 512],
                                start=(e == 0), stop=(e == DC - 1),
                            )
                        k_st = pa_st.tile([128, 512], BF16, tag="kst")
                        nc.scalar.activation(
                            k_st, k_ps, AF.Identity,
                            bias=bk_sb[:, d:d + 1], scale=1.0,
                        )
                        nc.sync.dma_start(
                            out=KtD[d * 128:(d + 1) * 128, nch * 512:(nch + 1) * 512],
                            in_=k_st,
                        )

                # V = H @ Wv^T + bv -> V_all [n-tile, dv] (SBUF resident)
                for nt in range(NT):
                    v_ps = ps_a.tile([128, DIM], F32, tag="a")
                    for e in range(DC):
                        nc.tensor.matmul(
                            v_ps,
                            Ht_all[:, e, nt * 128:(nt + 1) * 128],
                            Wv_sb[:, e, :],
                            start=(e == 0), stop=(e == DC - 1),
                        )
                    nc.vector.tensor_tensor(V_all[:, nt, :], v_ps, bv_bc, ALU.add)

            # ================= phase B: distance scale ==================
            with (
                tc.tile_pool(name="pb", bufs=1) as pb,
                tc.tile_pool(name="pb_st", bufs=3) as pb_st,
                tc.tile_pool(name="psum_b", bufs=3, space="PSUM") as ps_b,
            ):
                ttn_bc = pb.tile([128, N], F32, tag="ttnbc")
                Tts_all = pb.tile([128, DC, SHARD], BF16, tag="tts")
                tts_sb = pb.tile([128, SHARD // 128], F32, tag="ttssc")

                ttn_ap = ttn[:]
                nc.sync.dma_start(
                    out=ttn_bc,
                    in_=bass.AP(tensor=ttn_ap.tensor, offset=ttn_ap.offset,
                                ap=[[0, 128]] + ttn_ap.ap[1:]),
                )
                nc.sync.dma_start(out=Tts_all, in_=Tts[:].rearrange("(c p) n -> p c n", p=128))
                nc.sync.dma_start(out=tts_sb, in_=tts[:])

                MT = SHARD // 128  # 8 m tiles per core
                for nch in range(NCH):
                    tt_sl = pb_st.tile([128, DC, 512], BF16, tag="ttsl")
                    nc.sync.dma_start(
                        out=tt_sl,
                        in_=Tt[:, nch * 512:(nch + 1) * 512].rearrange(
                            "(c p) n -> p c n", p=128),
                    )
                    for mt in range(MT):
                        g_ps = ps_b.tile([128, 512], F32, tag="g")
                        for e in range(DC):
                            nc.tensor.matmul(
                                g_ps,
                                Tts_all[:, e, mt * 128:(mt + 1) * 128],
                                tt_sl[:, e, :],
                                start=(e == 0), stop=(e == DC - 1),
                            )
                        x_t = pb_st.tile([128, 512], F32, tag="x")
                        nc.vector.scalar_tensor_tensor(
                            x_t, g_ps, -2.0, ttn_bc[:, nch * 512:(nch + 1) * 512],
                            op0=ALU.mult, op1=ALU.add,
                        )
                        y_t = pb_st.tile([128, 512], F32, tag="y")
                        nc.vector.tensor_scalar(
                            y_t, x_t, tts_sb[:, mt:mt + 1], 1e-8,
                            op0=ALU.add, op1=ALU.max,
                        )
                        d_t = pb_st.tile([128, 512], BF16, tag="dst")
                        nc.scalar.activation(
                            d_t, y_t, AF.Sqrt, bias=0.0, scale=1.0,
                            accum_out=dist_acc[:, mt * NCH + nch: mt * NCH + nch + 1],
                        )

                # scale_col = 1 / (1 + mean) ; bounce to row layout via DRAM
                ds_sum = pb_st.tile([128, MBN * 4], F32, tag="dsum")
                nc.vector.tensor_reduce(
                    ds_sum,
                    dist_acc[:].rearrange("p (m t) -> p m t", t=NCH),
                    axis=AX.X, op=ALU.add,
                )
                sc_t = pb_st.tile([128, MBN * 4], F32, tag="sct")
                nc.vector.tensor_scalar(
                    sc_t, ds_sum, 1.0 / N, 1.0, op0=ALU.mult, op1=ALU.add,
                )
                nc.vector.reciprocal(scale_col, sc_t)
                nc.sync.dma_start(out=sscr[:].rearrange("t p -> p t"), in_=scale_col)
                nc.sync.dma_start(out=scale_row, in_=sscr[:].rearrange("t p -> () (t p)"))

            # ================= phase C: attention + projection ==========
            with (
                tc.tile_pool(name="pc", bufs=1) as pc,
                tc.tile_pool(name="pc_pt", bufs=3) as pc_pt,
                tc.tile_pool(name="pc_st", bufs=3) as pc_st,
                tc.tile_pool(name="psum_o", bufs=4, space="PSUM") as ps_o,
                tc.tile_pool(name="psum_d", bufs=1, space="PSUM") as ps_d,
                tc.tile_pool(name="psum_w", bufs=2, space="PSUM") as ps_w,
            ):
                Kt_all = pc.tile([128, DC, N], BF16, tag="kt")
                nc.sync.dma_start(out=Kt_all, in_=KtD[:].rearrange("(c p) n -> p c n", p=128))

                for mb in range(MBN):
                    o_ps = [ps_o.tile([128, MBS], F32, tag="o", name=f"ops{mb}_{i}")
                            for i in range(DC)]
                    den_ps = ps_d.tile([1, MBS], F32, tag="d")
                    pt_prev = None
                    for nt in range(NT):
                        s_ps = ps_w.tile([128, MBS], F32, tag="s")
                        for e in range(DC):
                            nc.tensor.matmul(
                                s_ps,
                                Kt_all[:, e, nt * 128:(nt + 1) * 128],
                                Qt_all[:, e, mb * MBS:(mb + 1) * MBS],
                                start=(e == 0), stop=(e == DC - 1),
                            )
                        # software pipeline: PV of previous tile first so PE
                        # never waits on the current exp
                        if pt_prev is not None:
                            pnt, ptile = pt_prev
                            for dv in range(DC):
                                nc.tensor.matmul(
                                    o_ps[dv],
                                    V_all[:, pnt, dv * 128:(dv + 1) * 128],
                                    ptile,
                                    start=(pnt == 0), stop=(pnt == NT - 1),
                                )
                            nc.tensor.matmul(
                                den_ps, ones_sb, ptile,
                                start=(pnt == 0), stop=(pnt == NT - 1),
                            )
                        p_t = pc_pt.tile([128, MBS], BF16, tag="pt")
                        nc.scalar.activation(p_t, s_ps, AF.Exp, bias=0.0,
                                             scale=float(INV_SQRT_D))
                        pt_prev = (nt, p_t)
                    pnt, ptile = pt_prev
                    for dv in range(DC):
                        nc.tensor.matmul(
                            o_ps[dv],
                            V_all[:, pnt, dv * 128:(dv + 1) * 128],
                            ptile,
                            start=(pnt == 0), stop=(pnt == NT - 1),
                        )
                    nc.tensor.matmul(den_ps, ones_sb, ptile,
                                     start=(pnt == 0), stop=(pnt == NT - 1))

                    # normalize: os = scale_s / denom, broadcast over partitions
                    rec_d = pc_st.tile([1, MBS], F32, tag="recd")
                    nc.vector.reciprocal(rec_d, den_ps)
                    os_row = pc_st.tile([1, MBS], F32, tag="osr")
                    nc.vector.tensor_tensor(
                        os_row, rec_d, scale_row[:, mb * MBS:(mb + 1) * MBS], ALU.mult)
                    nc.sync.dma_start(out=oscr, in_=os_row)
                    os_ap = oscr[:]
                    nc.sync.dma_start(
                        out=os_bc,
                        in_=bass.AP(tensor=os_ap.tensor, offset=os_ap.offset,
                                    ap=[[0, 128]] + os_ap.ap[1:]),
                    )

                    ot_sb = pc.tile([128, DC, MBS], BF16, tag=f"ot{mb}")
                    for dv in range(DC):
                        nc.vector.tensor_tensor(ot_sb[:, dv, :], o_ps[dv], os_bc, ALU.mult)

                    # Out^T = Wo @ O^T + bo
                    for e in range(DC):
                        p_ps = ps_w.tile([128, MBS], F32, tag="s")
                        for dv in range(DC):
                            nc.tensor.matmul(
                                p_ps,
                                WoT_all[:, dv, e * 128:(e + 1) * 128],
                                ot_sb[:, dv, :],
                                start=(dv == 0), stop=(dv == DC - 1),
                            )
                        out_st = pc_st.tile([128, MBS], F32, tag="outst")
                        nc.scalar.activation(out_st, p_ps, AF.Identity,
                                             bias=bo_sb[:, e:e + 1], scale=1.0)
                        nc.sync.dma_start(
                            out=OutT[e * 128:(e + 1) * 128, mb * MBS:(mb + 1) * MBS],
                            in_=out_st,
                        )

    nc.compile()
    return nc


def prepare_in_maps(H, T, Wq, bq, Wk, bk, Wv, bv, Wo, bo):
    H = np.asarray(H, np.float32)
    T = np.asarray(T, np.float32)
    Ht = np.ascontiguousarray(H.T).astype(bf16np)
    Tt = np.ascontiguousarray(T.T).astype(bf16np)
    tt = (T.astype(np.float64) ** 2).sum(axis=1).astype(np.float32)
    ttn_row = tt.reshape(1, N)

    def wT(W):
        return np.ascontiguousarray(np.asarray(W, np.float32).T).astype(bf16np)

    def bcol(b):
        return np.ascontiguousarray(
            np.asarray(b, np.float32).reshape(DC, 128).T)

    shared = {
        "Ht": Ht, "Tt": Tt, "ttn": ttn_row,
        "WqT": wT(Wq), "WkT": wT(Wk), "WvT": wT(Wv), "WoT": wT(Wo),
        "bq_c": bcol(bq), "bk_c": bcol(bk), "bo_c": bcol(bo),
        "bv_r": np.asarray(bv, np.float32).reshape(1, DIM).copy(),
    }
    in_maps = []
    for c in range(NCORES):
        sl = slice(c * SHARD, (c + 1) * SHARD)
        in_maps.append({
            **shared,
            "Hts": np.ascontiguousarray(Ht[:, sl]),
            "Tts": np.ascontiguousarray(Tt[:, sl]),
            "tts": np.ascontiguousarray(
                tt[sl].reshape(SHARD // 128, 128).T),
        })
    return in_maps


def run_on_hw(in_maps, trace=False):
    nc = build_kernel()
    res = bass_utils.run_bass_kernel_spmd(
        nc, in_maps, core_ids=list(range(NCORES)), trace=trace)
    return res


def kernel(H, T, Wq, bq, Wk, bk, Wv, bv, Wo, bo):
    in_maps = prepare_in_maps(H, T, Wq, bq, Wk, bk, Wv, bv, Wo, bo)
    res = run_on_hw(in_maps, trace=False)
    out = np.empty((N, DIM), np.float32)
    for c in range(NCORES):
        out[c * SHARD:(c + 1) * SHARD] = res.results[c]["OutT"].T
    return out


# revision 10
# speedup vs baseline: 1.0727x; 1.0727x over previous
"""Trainium2 Bass kernel for nn_CausalAttention_41961830482398.

Computes, for H,T [8192,512] and dim-512 linear layers Wq/Wk/Wv/Wo:
    dist  = pairwise_distances(T)                 # [N,N]
    scale = 1/(1 + mean(dist, axis=1))            # [N,1]
    Q,K,V = H@W{q,k,v}.T + b{q,k,v}
    attn  = softmax(Q@K.T / sqrt(512))
    out   = ((attn*scale) @ V) @ Wo.T + bo

Sharding: sequence-parallel over the row dim N across 8 cores (1024 rows
per core).  K, V and the projection weights are replicated (each core
computes full K,V from full H).  Everything is computed in a transposed
("S^T") layout so that no on-device transposes are needed:

  phase A: Kt = Wk@H^T + bk  [512,8192] (bounced via DRAM),
           V = H@Wv^T + bv   [8192,512] (SBUF-resident),
           Qt = Wq@Hs^T + bq [512,1024]
  phase B: G = Ts@T^T, dist = sqrt(max(tts+ttn-2G,0)+1e-8),
           row-mean via ACT accum -> scale_s  [1024]
  phase C: S^T tile = K@Qs^T  [n=128, m=512]; Pt = exp(S^T/sqrt(d));
           O^T += V^T@P^T via lhsT=V tiles; denom via ones-matmul;
           projection: Out^T = (Wo@O^T)*(scale_s/denom) + bo -> DRAM.

Host passes pre-transposed/bf16-cast inputs and per-core shard slices;
the kernel returns Out^T per core which the host gathers + transposes.
"""

import numpy as np
import ml_dtypes

import concourse.bass as bass
import concourse.mybir as mybir
import concourse.tile as tile
from concourse import bacc
from concourse import bass_utils

N, DIM = 8192, 512
NCORES = 8
SHARD = N // NCORES          # 1024 rows per core
DC = DIM // 128              # 4 contraction chunks
NT = N // 128                # 64 key tiles
MBS = 512                    # m free-dim block
MBN = SHARD // MBS           # 2 m-blocks
NCH = N // 512               # 16 n chunks of 512
NBLK = N // 2048             # 4 wide blocks for dist
MT = SHARD // 128            # 8 m tiles per core
INV_SQRT_D = 1.0 / np.sqrt(np.float32(DIM))

BF16 = mybir.dt.bfloat16
F32 = mybir.dt.float32
AF = mybir.ActivationFunctionType
ALU = mybir.AluOpType
AX = mybir.AxisListType

bf16np = ml_dtypes.bfloat16


def _bcast_rows(ap, p=128):
    """DRAM row [1, n] -> broadcast AP [[0,p], [1,n]] for DMA replication."""
    return bass.AP(tensor=ap.tensor, offset=ap.offset, ap=[[0, p]] + ap.ap[1:])


def build_kernel():
    nc = bacc.Bacc("TRN2", target_bir_lowering=False, debug=False)

    # ---- DRAM I/O ------------------------------------------------------
    Ht = nc.dram_tensor("Ht", [DIM, N], BF16, kind="ExternalInput")
    Hts = nc.dram_tensor("Hts", [DIM, SHARD], BF16, kind="ExternalInput")
    Tt = nc.dram_tensor("Tt", [DIM, N], BF16, kind="ExternalInput")
    Tts = nc.dram_tensor("Tts", [DIM, SHARD], BF16, kind="ExternalInput")
    ttn = nc.dram_tensor("ttn", [1, N], F32, kind="ExternalInput")
    tts = nc.dram_tensor("tts", [128, MT], F32, kind="ExternalInput")
    WqT = nc.dram_tensor("WqT", [DIM, DIM], BF16, kind="ExternalInput")
    WkT = nc.dram_tensor("WkT", [DIM, DIM], BF16, kind="ExternalInput")
    WvT = nc.dram_tensor("WvT", [DIM, DIM], BF16, kind="ExternalInput")
    WoT = nc.dram_tensor("WoT", [DIM, DIM], BF16, kind="ExternalInput")
    bq_c = nc.dram_tensor("bq_c", [128, DC], F32, kind="ExternalInput")
    bk_c = nc.dram_tensor("bk_c", [128, DC], F32, kind="ExternalInput")
    bo_c = nc.dram_tensor("bo_c", [128, DC], F32, kind="ExternalInput")
    bv_r = nc.dram_tensor("bv_r", [1, DIM], F32, kind="ExternalInput")
    OutT = nc.dram_tensor("OutT", [DIM, SHARD], F32, kind="ExternalOutput")

    with tile.TileContext(nc) as tc:
        with (
            tc.tile_pool(name="dram", bufs=1, space="DRAM") as dpool,
            tc.tile_pool(name="singles", bufs=1) as sg,
        ):
            KtD = dpool.tile([DIM, N], BF16, tag="ktd")
            sscr = dpool.tile([MT, 128], F32, tag="sscr")
            oscr = dpool.tile([MBN, MBS], F32, tag="oscr")

            # long-lived SBUF tensors
            V_all = sg.tile([128, NT, DIM], BF16, tag="v")       # 64KB/part
            Qt_all = sg.tile([128, DC, SHARD], BF16, tag="qt")   # 8KB
            WoT_all = sg.tile([128, DC, DIM], BF16, tag="wot")   # 4KB
            bo_sb = sg.tile([128, DC], F32, tag="bo")
            ones_sb = sg.tile([128, 1], BF16, tag="ones")
            dist_acc = sg.tile([128, MT * NBLK], F32, tag="dacc")
            scale_col = sg.tile([128, MT], F32, tag="scol")
            scale_row = sg.tile([1, SHARD], F32, tag="srow")

            nc.vector.memset(ones_sb, 1.0)

            # ================= phase A: projections =====================
            with (
                tc.tile_pool(name="pa", bufs=1) as pa,
                tc.tile_pool(name="pa_st", bufs=4) as pa_st,
                tc.tile_pool(name="psum_a", bufs=3, space="PSUM") as ps_a,
            ):
                Hts_all = pa.tile([128, DC, SHARD], BF16, tag="hts")
                Wq_sb = pa.tile([128, DC, DIM], BF16, tag="wq")
                Wk_sb = pa.tile([128, DC, DIM], BF16, tag="wk")
                Wv_sb = pa.tile([128, DC, DIM], BF16, tag="wv")
                bq_sb = pa.tile([128, DC], F32, tag="bq")
                bk_sb = pa.tile([128, DC], F32, tag="bk")
                bv_bc = pa.tile([128, DIM], F32, tag="bvbc")
                Ht_all = pa.tile([128, DC, N], BF16, tag="ht")

                # small loads first so Qt matmuls start the PE early
                nc.sync.dma_start(out=Wq_sb, in_=WqT[:].rearrange("(c p) d -> p c d", p=128))
                nc.sync.dma_start(out=Hts_all, in_=Hts[:].rearrange("(c p) n -> p c n", p=128))
                nc.sync.dma_start(out=bq_sb, in_=bq_c[:])
                nc.sync.dma_start(out=Wk_sb, in_=WkT[:].rearrange("(c p) d -> p c d", p=128))
                nc.sync.dma_start(out=Wv_sb, in_=WvT[:].rearrange("(c p) d -> p c d", p=128))
                nc.sync.dma_start(out=bk_sb, in_=bk_c[:])
                nc.sync.dma_start(out=bv_bc, in_=_bcast_rows(bv_r[:]))
                nc.sync.dma_start(out=WoT_all, in_=WoT[:].rearrange("(c p) d -> p c d", p=128))
                nc.sync.dma_start(out=bo_sb, in_=bo_c[:])
                # Ht streamed in 16 column blocks so Kt/V start early
                for nch in range(NCH):
                    nc.sync.dma_start(
                        out=Ht_all[:, :, nch * 512:(nch + 1) * 512],
                        in_=Ht[:, nch * 512:(nch + 1) * 512].rearrange(
                            "(c p) n -> p c n", p=128),
                    )

                # Qt = Wq @ Hs^T + bq   -> Qt_all [d-chunk, m]
                for d in range(DC):
                    for mb in range(MBN):
                        q_ps = ps_a.tile([128, MBS], F32, tag="a")
                        for e in range(DC):
                            nc.tensor.matmul(
                                q_ps,
                                Wq_sb[:, e, d * 128:(d + 1) * 128],
                                Hts_all[:, e, mb * MBS:(mb + 1) * MBS],
                                start=(e == 0), stop=(e == DC - 1),
                            )
                        nc.scalar.activation(
                            Qt_all[:, d, mb * MBS:(mb + 1) * MBS], q_ps,
                            AF.Identity, bias=bq_sb[:, d:d + 1], scale=1.0,
                        )

                # per Ht block: Kt column block (4 d-strips) + 4 V tiles
                for nch in range(NCH):
                    for d in range(DC):
                        k_ps = ps_a.tile([128, 512], F32, tag="a")
                        for e in range(DC):
                            nc.tensor.matmul(
                                k_ps,
                                Wk_sb[:, e, d * 128:(d + 1) * 128],
                                Ht_all[:, e, nch * 512:(nch + 1) * 512],
                                start=(e == 0), stop=(e == DC - 1),
                            )
                        k_st = pa_st.tile([128, 512], BF16, tag="kst")
                        nc.scalar.activation(
                            k_st, k_ps, AF.Identity,
                            bias=bk_sb[:, d:d + 1], scale=1.0,
                        )
                        nc.sync.dma_start(
                            out=KtD[d * 128:(d + 1) * 128, nch * 512:(nch + 1) * 512],
                            in_=k_st,
                        )
                    for nt in range(nch * 4, nch * 4 + 4):
                        v_ps = ps_a.tile([128, DIM], F32, tag="a")
                        for e in range(DC):
                            nc.tensor.matmul(
                                v_ps,
                                Ht_all[:, e, nt * 128:(nt + 1) * 128],
                                Wv_sb[:, e, :],
                                start=(e == 0), stop=(e == DC - 1),
                            )
                        nc.vector.tensor_tensor(V_all[:, nt, :], v_ps, bv_bc, ALU.add)

            # ================= phase B: distance scale ==================
            with (
                tc.tile_pool(name="pb", bufs=1) as pb,
                tc.tile_pool(name="pb_tt", bufs=2) as pb_tt,
                tc.tile_pool(name="pb_st", bufs=3) as pb_st,
                tc.tile_pool(name="psum_b", bufs=3, space="PSUM") as ps_b,
            ):
                ttn_bc = pb.tile([128, N], F32, tag="ttnbc")
                Tts_all = pb.tile([128, DC, SHARD], BF16, tag="tts")
                tts_sb = pb.tile([128, MT], F32, tag="ttssc")

                nc.sync.dma_start(out=Tts_all, in_=Tts[:].rearrange("(c p) n -> p c n", p=128))
                nc.sync.dma_start(out=tts_sb, in_=tts[:])
                nc.sync.dma_start(out=ttn_bc, in_=_bcast_rows(ttn[:]))

                for blk in range(NBLK):
                    b0 = blk * 2048
                    tt_sl = pb_tt.tile([128, DC, 2048], BF16, tag="ttsl")
                    nc.sync.dma_start(
                        out=tt_sl,
                        in_=Tt[:, b0:b0 + 2048].rearrange("(c p) n -> p c n", p=128),
                    )
                    for mt in range(MT):
                        x_t = pb_st.tile([128, 2048], F32, tag="x")
                        for sub in range(4):
                            g_ps = ps_b.tile([128, 512], F32, tag="g")
                            for e in range(DC):
                                nc.tensor.matmul(
                                    g_ps,
                                    Tts_all[:, e, mt * 128:(mt + 1) * 128],
                                    tt_sl[:, e, sub * 512:(sub + 1) * 512],
                                    start=(e == 0), stop=(e == DC - 1),
                                )
                            nc.vector.scalar_tensor_tensor(
                                x_t[:, sub * 512:(sub + 1) * 512], g_ps, -2.0,
                                ttn_bc[:, b0 + sub * 512:b0 + (sub + 1) * 512],
                                op0=ALU.mult, op1=ALU.add,
                            )
                        nc.vector.tensor_scalar(
                            x_t, x_t, tts_sb[:, mt:mt + 1], 1e-8,
                            op0=ALU.add, op1=ALU.max,
                        )
                        d_t = pb_st.tile([128, 2048], BF16, tag="dst")
                        nc.scalar.activation(
                            d_t, x_t, AF.Sqrt, bias=0.0, scale=1.0,
                            accum_out=dist_acc[:, mt * NBLK + blk: mt * NBLK + blk + 1],
                        )

                # scale_col = 1 / (1 + mean) ; bounce to row layout via DRAM
                ds_sum = pb_st.tile([128, MT], F32, tag="dsum")
                nc.vector.tensor_reduce(
                    ds_sum,
                    dist_acc[:].rearrange("p (m t) -> p m t", t=NBLK),
                    axis=AX.X, op=ALU.add,
                )
                sc_t = pb_st.tile([128, MT], F32, tag="sct")
                nc.vector.tensor_scalar(
                    sc_t, ds_sum, 1.0 / N, 1.0, op0=ALU.mult, op1=ALU.add,
                )
                nc.vector.reciprocal(scale_col, sc_t)
                nc.sync.dma_start(out=sscr[:].rearrange("t p -> p t"), in_=scale_col)
                nc.sync.dma_start(out=scale_row, in_=sscr[:].rearrange("t p -> () (t p)"))

            # ================= phase C: attention + projection ==========
            with (
                tc.tile_pool(name="pc", bufs=1) as pc,
                tc.tile_pool(name="pc_pt", bufs=3) as pc_pt,
                tc.tile_pool(name="pc_st", bufs=3) as pc_st,
                tc.tile_pool(name="psum_o", bufs=4, space="PSUM") as ps_o,
                tc.tile_pool(name="psum_d", bufs=2, space="PSUM") as ps_d,
                tc.tile_pool(name="psum_w", bufs=2, space="PSUM") as ps_w,
            ):
                Kt_all = pc.tile([128, DC, N], BF16, tag="kt")
                for nch in range(NCH):
                    nc.sync.dma_start(
                        out=Kt_all[:, :, nch * 512:(nch + 1) * 512],
                        in_=KtD[:, nch * 512:(nch + 1) * 512].rearrange(
                            "(c p) n -> p c n", p=128),
                    )

                ot_sb = pc.tile([128, MBN, DC, MBS], BF16, tag="ot")
                os_bc = pc.tile([128, MBN, MBS], F32, tag="osbc")

                # -- attention passes (back to back so PE stays dense) --
                for mb in range(MBN):
                    o_ps = [ps_o.tile([128, MBS], F32, tag="o", name=f"ops{mb}_{i}")
                            for i in range(DC)]
                    den_ps = ps_d.tile([1, MBS], F32, tag="d")
                    pt_prev = None
                    for nt in range(NT):
                        s_ps = ps_w.tile([128, MBS], F32, tag="s")
                        for e in range(DC):
                            nc.tensor.matmul(
                                s_ps,
                                Kt_all[:, e, nt * 128:(nt + 1) * 128],
                                Qt_all[:, e, mb * MBS:(mb + 1) * MBS],
                                start=(e == 0), stop=(e == DC - 1),
                            )
                        # software pipeline: PV of previous tile first so PE
                        # never waits on the current exp
                        if pt_prev is not None:
                            pnt, ptile = pt_prev
                            for dv in range(DC):
                                nc.tensor.matmul(
                                    o_ps[dv],
                                    V_all[:, pnt, dv * 128:(dv + 1) * 128],
                                    ptile,
                                    start=(pnt == 0), stop=(pnt == NT - 1),
                                )
                            nc.tensor.matmul(
                                den_ps, ones_sb, ptile,
                                start=(pnt == 0), stop=(pnt == NT - 1),
                            )
                        p_t = pc_pt.tile([128, MBS], BF16, tag="pt")
                        nc.scalar.activation(p_t, s_ps, AF.Exp, bias=0.0,
                                             scale=float(INV_SQRT_D))
                        pt_prev = (nt, p_t)
                    pnt, ptile = pt_prev
                    for dv in range(DC):
                        nc.tensor.matmul(
                            o_ps[dv],
                            V_all[:, pnt, dv * 128:(dv + 1) * 128],
                            ptile,
                            start=(pnt == 0), stop=(pnt == NT - 1),
                        )
                    nc.tensor.matmul(den_ps, ones_sb, ptile,
                                     start=(pnt == 0), stop=(pnt == NT - 1))

                    # raw O^T copies (frees psum for the next m-block)
                    for dv in range(DC):
                        nc.vector.tensor_copy(ot_sb[:, mb, dv, :], o_ps[dv])

                    # os = scale_s/denom -> row -> broadcast via DRAM bounce
                    # (overlaps with the next attention pass / projections)
                    rec_d = pc_st.tile([1, MBS], F32, tag="recd")
                    nc.vector.reciprocal(rec_d, den_ps)
                    os_row = pc_st.tile([1, MBS], F32, tag="osr")
                    nc.vector.tensor_tensor(
                        os_row, rec_d, scale_row[:, mb * MBS:(mb + 1) * MBS], ALU.mult)
                    nc.sync.dma_start(out=oscr[mb:mb + 1, :], in_=os_row)
                    nc.sync.dma_start(out=os_bc[:, mb, :],
                                      in_=_bcast_rows(oscr[mb:mb + 1, :]))

                # -- projections: Out^T = (Wo @ O^T) * os + bo --
                for mb in range(MBN):
                    for e in range(DC):
                        p_ps = ps_w.tile([128, MBS], F32, tag="s")
                        for dv in range(DC):
                            nc.tensor.matmul(
                                p_ps,
                                WoT_all[:, dv, e * 128:(e + 1) * 128],
                                ot_sb[:, mb, dv, :],
                                start=(dv == 0), stop=(dv == DC - 1),
                            )
                        sc_st = pc_st.tile([128, MBS], F32, tag="scst")
                        nc.vector.tensor_tensor(sc_st, p_ps, os_bc[:, mb, :], ALU.mult)
                        out_st = pc_st.tile([128, MBS], F32, tag="outst")
                        nc.scalar.activation(out_st, sc_st, AF.Identity,
                                             bias=bo_sb[:, e:e + 1], scale=1.0)
                        nc.sync.dma_start(
                            out=OutT[e * 128:(e + 1) * 128, mb * MBS:(mb + 1) * MBS],
                            in_=out_st,
                        )

    nc.compile()
    return nc


def prepare_in_maps(H, T, Wq, bq, Wk, bk, Wv, bv, Wo, bo):
    H = np.asarray(H, np.float32)
    T = np.asarray(T, np.float32)
    Ht = np.ascontiguousarray(H.T).astype(bf16np)
    Tt = np.ascontiguousarray(T.T).astype(bf16np)
    tt = (T.astype(np.float64) ** 2).sum(axis=1).astype(np.float32)
    ttn_row = tt.reshape(1, N)

    def wT(W):
        return np.ascontiguousarray(np.asarray(W, np.float32).T).astype(bf16np)

    def bcol(b):
        return np.ascontiguousarray(
            np.asarray(b, np.float32).reshape(DC, 128).T)

    shared = {
        "Ht": Ht, "Tt": Tt, "ttn": ttn_row,
        "WqT": wT(Wq), "WkT": wT(Wk), "WvT": wT(Wv), "WoT": wT(Wo),
        "bq_c": bcol(bq), "bk_c": bcol(bk), "bo_c": bcol(bo),
        "bv_r": np.asarray(bv, np.float32).reshape(1, DIM).copy(),
    }
    in_maps = []
    for c in range(NCORES):
        sl = slice(c * SHARD, (c + 1) * SHARD)
        in_maps.append({
            **shared,
            "Hts": np.ascontiguousarray(Ht[:, sl]),
            "Tts": np.ascontiguousarray(Tt[:, sl]),
            "tts": np.ascontiguousarray(
                tt[sl].reshape(MT, 128).T),
        })
    return in_maps


def run_on_hw(in_maps, trace=False):
    nc = build_kernel()
    res = bass_utils.run_bass_kernel_spmd(
        nc, in_maps, core_ids=list(range(NCORES)), trace=trace)
    return res


def kernel(H, T, Wq, bq, Wk, bk, Wv, bv, Wo, bo):
    in_maps = prepare_in_maps(H, T, Wq, bq, Wk, bk, Wv, bv, Wo, bo)
    res = run_on_hw(in_maps, trace=False)
    out = np.empty((N, DIM), np.float32)
    for c in range(NCORES):
        out[c * SHARD:(c + 1) * SHARD] = res.results[c]["OutT"].T
    return out
